# revision 6
# baseline (speedup 1.0000x reference)
"""Trainium2 Bass kernel for nn_MemoryReader.

Reference computation (per batch b):
    mi = mk.reshape(CK, N);  qi = qk.reshape(CK, P) / sqrt(CK)
    S  = mi.T @ qi                      # [N, P] affinity logits
    A  = softmax(S, axis=0)             # over memory axis N
    mem = mv.reshape(CV, N) @ A         # [CV, P]
    out = concat([mem, qv], axis=channel)

Sharding: 8 cores = (4 batches) x (2 halves of the memory axis N).
Each core computes, for its (b, half):
    S      = mk.T @ qk                          # fp8 operands, f32 PSUM
    E      = exp(0.125*S - 2)                   # fp8e4; the -2 bias keeps
                                                # E<=54 (TRN fp8e4 max 240)
                                                # and cancels in the softmax
    memT   = E.T @ [mvT | 1]                    # [P, 513]: col 512 = sum(E)
The host combines: mem = (num_0 + num_1) / (den_0 + den_1), then concats
qv (pure passthrough). No on-device collectives needed.

Speed features (measured lineage: 210.7us bf16 -> 131.4us fp8-DR -> this):
  - mm1 (CK=64 contraction) uses PE row tiling (64x128 mode): mk tile
    pairs are stacked in SBUF partitions (rows 0:64 = even n-tile,
    64:128 = odd), qk is duplicated into partitions 64:128, and the two
    matmuls of a pair run CONCURRENTLY on array tiles T0/T8. A
    post-build pass widens T0's LDWEIGHTS to all 128 rows and deletes
    T8's, so each pair does one full-row (background/FWL-eligible)
    weight load. Concurrent row tiles must write different PSUM banks,
    so pair outputs land in a 2-bank "s4" squad tile [128, 2(bank),
    2(pair), 256].
  - mm2 (89% of FLOPs) runs in fp8e4 DoubleRow mode: each matmul
    contracts TWO 128-row n-tiles (lhsT/rhs get [K, 2, M] APs).
    e4 is laid out [128, 2(b), 2(i), 256] so a DR pair is the
    strided slice e4[:, :, i, :] (Ko step 512B).
  - The softmax denominator is a 513th "ones" column of mvT, accumulated
    by the same mm2 matmuls (split 256+257 to satisfy the one-PSUM-bank
    rule, sharing one LDWEIGHTS via a post-build dedupe pass).
  - exp() is fused 4 n-tiles per ACT instruction (reads the whole s4
    squad across its 2 banks) to amortize the ~185ns ACT access latency;
    ACT stays just under the PE's per-unit time.
  - acc_a for the chunk's two p-slices shares ONE PSUM bank ([128, 2,
    256]); slice 1's first matmul uses start=False (slice 0's start
    already cleared the bank's has_written bits) - frees the bank needed
    by the s4 double buffer.
  - All inputs ship as fp8 in few, fat, consumption-ordered DMAs (2KB+
    per-partition rows; sub-2KB rows run ~5x slower). The paired mk
    layout halves mk bytes vs the old zero-padded K=128 layout.
  - A flat (chunk, quad) software pipeline issues unit u+1's mm1 before
    unit u's mm2, keeping the PE queue gapless.
"""

import numpy as np
import ml_dtypes

import concourse.tile as tile
from concourse import bacc, mybir
from concourse.bass_utils import run_bass_kernel_spmd

# Problem shape (hardcoded per contract)
B, CK, CV, T, H, W = 4, 64, 512, 8, 30, 54
N = T * H * W          # 12960 memory positions
P = H * W              # 1620 query positions
NHALF = N // 2         # 6480 per core
NT = (NHALF + 127) // 128   # 51 n-tiles (last has 80 rows)
NLAST = NHALF - (NT - 1) * 128  # 80
NB = (NT + 1) // 2     # 26 mk blocks (tile pair per block; tile 51 = pad)
NTP = 2 * NB           # 52 mvT tiles (pad tile NT..NTP zero)
MVW = 528              # mvT free width: 512 mv + 1 ones + 15 pad (16B align)
CVA = 513              # real mm2 output width (512 mv + 1 denominator)
ASPL = 256             # a-half columns (b-half = 257); each fits a PSUM bank
EXP_BIAS = -2.0        # exp(0.125*s - 2): range safety for fp8e4

# p-axis chunks of 256 (2 slices of 128 each; 84-wide remainder last — it
# has the smallest output-DMA tail).
PCH = [(0, 256), (256, 256), (512, 256), (768, 256), (1024, 256),
       (1280, 256), (1536, 84)]
QUADS = [(0, 4), (4, 4), (8, 4), (12, 4), (16, 4), (20, 4), (24, 4), (28, 4),
         (32, 4), (36, 4), (40, 4), (44, 4), (48, 3)]

DEDUPE_LDW = True      # share one LDW between mm2's a/b column halves
MERGE_MM1_LDW = True   # one full-row LDW per mm1 pair (else per-half LDWs)
N_WARMUP = 28

_CACHE = {}


def _build_program():
    f8 = mybir.dt.float8e4
    bf16 = mybir.dt.bfloat16
    f32 = mybir.dt.float32
    DR = mybir.MatmulPerfMode.DoubleRow
    nc = bacc.Bacc(None, target_bir_lowering=False, debug=False)

    # mk blocks: partitions 0:64 = even n-tile's CK channels, 64:128 = odd
    # n-tile's. Half the bytes of the old zero-padded K=128 layout, and the
    # shape row tiling wants.
    mk_d = nc.declare_dram_parameter("mk", [128, NB, 128], f8, isOutput=False)
    # qk duplicated into partitions 64:128 (the T8 row tile streams from
    # there). Padded to 2048 cols so the one transfer has 2KB rows.
    qk_d = nc.declare_dram_parameter("qk", [128, 2048], f8, isOutput=False)
    mvt_d = nc.declare_dram_parameter("mvT", [128, NTP, MVW], f8, isOutput=False)
    mem_d = nc.declare_dram_parameter("memT", [P, CVA], f32, isOutput=True)

    with tile.TileContext(nc) as tc:
        with (
            tc.tile_pool(name="singles", bufs=1) as singles,
            tc.tile_pool(name="epool", bufs=3) as epool,
            tc.tile_pool(name="opool", bufs=4) as opool,
            tc.tile_pool(name="spsum", bufs=2, space="PSUM") as spsum,
            tc.tile_pool(name="apsum", bufs=2, space="PSUM") as apsum,
            tc.tile_pool(name="bpsum", bufs=2, space="PSUM") as bpsum,
        ):
            qk_sb = singles.tile([128, 2048], f8)
            mk_sb = singles.tile([128, NB, 128], f8)
            mvt_sb = singles.tile([128, NTP, MVW], f8)
            # DMA facts (measured): every transfer stripes over all 16
            # queues and completes in ISSUE ORDER at aggregate BW, but
            # sub-2KB per-partition rows run ~5x slower (per-row descriptor
            # overhead), and each trigger costs ~660ns on the Sync queue.
            # So: few, fat, consumption-ordered transfers. mk is now one
            # 3.3KB-row transfer (the paired layout has no zero half).
            nc.sync.dma_start(out=qk_sb[:, :], in_=qk_d[:, :])
            nc.sync.dma_start(out=mk_sb[:, :, :], in_=mk_d[:, :, :])
            mv_groups = [(g, min(g + 4, NTP)) for g in range(0, NTP, 4)]
            for g0, g1 in mv_groups:
                nc.sync.dma_start(out=mvt_sb[:, g0:g1, :],
                                  in_=mvt_d[:, g0:g1, :])

            # Warm-up: full-size matmuls on a memset tile, depending on no
            # DMA. They fill the initial PE idle gap AND release the HAM
            # clock throttle (~3.4us of sustained activity needed).
            warmw = singles.tile([128, 128], bf16, name="warmw")
            nc.vector.memset(warmw, 1.0)
            bias_sb = singles.tile([128, 1], f32, name="bias")
            nc.vector.memset(bias_sb, EXP_BIAS)
            warm = spsum.tile([128, 2, 2, ASPL], f32, tag="s4", name="warm")
            for _ in range(N_WARMUP):
                nc.tensor.matmul(warm[:, 0, 0, 0:128], lhsT=warmw, rhs=warmw,
                                 start=True, stop=True)

            # Flat software pipeline over (chunk, quad) units: issue unit
            # u+1's mm1 before unit u's mm2 so the PE queue never stalls on
            # the ACT->mm2 dependency.
            units = [(ci, qi) for ci in range(len(PCH)) for qi in range(len(QUADS))]
            squads = {}
            e4s = {}
            accs = {}

            def issue_mm1(u):
                ci, qi = units[u]
                ps, w = PCH[ci]
                q0, qn = QUADS[qi]
                # s4 squad tile: [128, bank b, pair i, 256]. Pair i covers
                # n-tiles (q0+2i, q0+2i+1): the even tile runs on array row
                # tile T0 (SBUF partitions 0:64) into bank 0, the odd on T8
                # (partitions 64:128) into bank 1 — concurrent row tiles
                # must hit different PSUM banks.
                s4 = spsum.tile([128, 2, 2, ASPL], f32, tag="s4", name="s4")
                b0 = q0 // 2
                for i in range(2):
                    blk = b0 + i
                    nt_even = q0 + 2 * i
                    if nt_even >= NT:
                        break
                    nsz = 128 if nt_even < NT - 1 else NLAST
                    nc.tensor.matmul(
                        s4[:nsz, 0, i, :w],
                        lhsT=mk_sb[0:64, blk, :nsz],
                        rhs=qk_sb[0:64, ps:ps + w],
                        start=True,
                        stop=True,
                    )
                    if nt_even + 1 < NT:
                        nc.tensor.matmul(
                            s4[:128, 1, i, :w],
                            lhsT=mk_sb[64:128, blk, :128],
                            rhs=qk_sb[64:128, ps:ps + w],
                            start=True,
                            stop=True,
                        )
                squads[u] = s4

            def issue_act(u):
                ci, qi = units[u]
                ps, w = PCH[ci]
                q0, qn = QUADS[qi]
                # e4[p, b, i, c] = exp of n-tile q0 + 2i + b. One fused ACT
                # per quad: reads the s4 squad across both banks (linear
                # in/out APs). For the 3-tile last quad, slot (1,1) is exp
                # of stale PSUM — mm2 never reads it (same trick as rows
                # 80:128 of tile 50).
                e4 = epool.tile([128, 2, 2, ASPL], f8, tag="e", name="e")
                s4 = squads.pop(u)
                nc.scalar.activation(
                    out=e4[:, :, :, :w],
                    in_=s4[:, :, :, :w],
                    func=mybir.ActivationFunctionType.Exp,
                    scale=0.125,  # 1/sqrt(CK)
                    bias=bias_sb[:, :],
                )
                e4s[u] = e4

            def issue_mm2(u):
                ci, qi = units[u]
                ps, w = PCH[ci]
                q0, qn = QUADS[qi]
                e4 = e4s.pop(u)
                nslices = (w + 127) // 128
                if qi == 0:
                    # acc_a: ONE bank for both p-slices ([128, 2, 256]).
                    # acc_b: 257 cols per slice won't pair in a bank.
                    accs[ci] = (
                        apsum.tile([128, 2, ASPL], f32, tag="acc_a",
                                   name="acc_a"),
                        [bpsum.tile([128, 512], f32, tag="acc_b", name="acc_b")
                         for _ in range(nslices)],
                    )
                first = qi == 0
                last = qi == len(QUADS) - 1

                def mm2_step(i, dr, sl, st, sp):
                    nt = q0 + 2 * i
                    nsz = 128 if dr else NLAST
                    pw = min(128, w - 128 * sl)
                    acc_a, acc_bs = accs[ci]
                    if dr:
                        el = e4[:nsz, :, i, sl * 128:sl * 128 + pw]
                        nc.tensor.matmul(
                            acc_a[:pw, sl, 0:ASPL],
                            lhsT=el,
                            rhs=mvt_sb[:nsz, nt:nt + 2, 0:ASPL],
                            # slice 1 shares slice 0's bank: slice 0's
                            # start already cleared the bank's has_written
                            # bits, so slice 1 must NOT clear them again.
                            start=st and sl == 0, stop=sp,
                            perf_mode=DR,
                        )
                        nc.tensor.matmul(
                            acc_bs[sl][:pw, 0:CVA - ASPL],
                            lhsT=el,
                            rhs=mvt_sb[:nsz, nt:nt + 2, ASPL:CVA],
                            start=st, stop=sp,
                            perf_mode=DR,
                        )
                    else:
                        el = e4[:nsz, 0, i, sl * 128:sl * 128 + pw]
                        nc.tensor.matmul(
                            acc_a[:pw, sl, 0:ASPL],
                            lhsT=el,
                            rhs=mvt_sb[:nsz, nt, 0:ASPL],
                            start=st and sl == 0, stop=sp,
                        )
                        nc.tensor.matmul(
                            acc_bs[sl][:pw, 0:CVA - ASPL],
                            lhsT=el,
                            rhs=mvt_sb[:nsz, nt, ASPL:CVA],
                            start=st, stop=sp,
                        )

                def copy_out(sl, on_act):
                    pw = min(128, w - 128 * sl)
                    acc_a, acc_bs = accs[ci]
                    o_sb = opool.tile([128, CVA], f32, tag="o", name="o")
                    nc.vector.tensor_copy(out=o_sb[:pw, 0:ASPL],
                                          in_=acc_a[:pw, sl, 0:ASPL])
                    if on_act:
                        # ACT is idle at the kernel tail; mid-stream ACT
                        # copies delay the next chunk's exp.
                        nc.scalar.activation(
                            out=o_sb[:pw, ASPL:CVA],
                            in_=acc_bs[sl][:pw, 0:CVA - ASPL],
                            func=mybir.ActivationFunctionType.Copy,
                        )
                    else:
                        nc.vector.tensor_copy(out=o_sb[:pw, ASPL:CVA],
                                              in_=acc_bs[sl][:pw, 0:CVA - ASPL])
                    p0 = ps + sl * 128
                    nc.sync.dma_start(out=mem_d[p0:p0 + pw, :],
                                      in_=o_sb[:pw, :])

                # DR pairs over (i=0: tiles q0,q0+1) and (i=1: q0+2,q0+3);
                # the 3-tile last quad does DR(48,49) then single(50).
                if qn == 4:
                    steps = [(0, True), (1, True)]
                else:
                    steps = [(0, True), (1, False)]
                for si, (i, dr) in enumerate(steps):
                    st = first and si == 0
                    sp = last and si == len(steps) - 1
                    for sl in range(nslices):
                        mm2_step(i, dr, sl, st, sp)
                if last:
                    final = ci == len(PCH) - 1
                    for sl in range(nslices):
                        copy_out(sl, on_act=final)
                    del accs[ci]

            issue_mm1(0)
            for u in range(len(units)):
                if u + 1 < len(units):
                    issue_mm1(u + 1)
                issue_act(u)
                issue_mm2(u)

    _strip_same_engine_waits(nc)
    if MERGE_MM1_LDW:
        _merge_mm1_pair_ldweights(nc)
    if DEDUPE_LDW:
        _dedupe_ldweights(nc)
    nc.compile()
    return nc


def _merge_waits(keep, dropped):
    si = getattr(dropped, "sync_info", None)
    if si is not None and si.on_wait:
        ksi = keep.sync_info
        if ksi is None:
            keep.sync_info = si
        else:
            have = {repr(w) for w in ksi.on_wait}
            for w_ in si.on_wait:
                if repr(w_) not in have:
                    ksi.on_wait.append(w_)
        assert not (si.on_update or []), "dropped LDW had sem updates"


def _merge_mm1_pair_ldweights(nc):
    """Fuse each mm1 pair's two half-row LDWEIGHTS into one full-row load.

    The legalizer emits, per pair: LDW(T0: partitions 0:64) MM(T0)
    LDW(T8: partitions 64:128) MM(T8). The T0 AP starts at partition 0,
    so widening its partition count to 128 makes it load the whole mk
    block — T0's rows land in array rows 0:63, T8's in 64:127 — and the
    T8 load becomes redundant. A full-row load is also what the weight
    port overlaps best (background buffer / FWL). T8's waits move to the
    surviving (earlier) load."""
    for fn in nc.m.functions:
        for blk in fn.blocks:
            keep = []
            pending = None  # the T0 LDW awaiting its T8 partner
            for inst in blk.instructions:
                if isinstance(inst, mybir.InstLdweights):
                    ts = getattr(inst, "tile_size", None)
                    tp = getattr(inst, "tile_position", None)
                    if ts == (64, 128) and tp == (0, 0):
                        pending = inst
                        keep.append(inst)
                        continue
                    if (ts == (64, 128) and tp == (64, 0)
                            and pending is not None):
                        ap = pending.ins[0]
                        ap8 = inst.ins[0]
                        dims = [list(d) for d in ap.ap]
                        # same mk block: T8 = T0 shifted 64 partitions
                        assert dims[0][1] == 64, dims
                        assert ap8.memref == ap.memref
                        assert ap8.offset == ap.offset + 64 * dims[0][0], (
                            ap8.offset, ap.offset, dims)
                        dims[0][1] = 128
                        ap.ap = dims
                        pending.tile_size = (128, 128)
                        _merge_waits(pending, inst)
                        pending = None
                        continue
                    pending = None
                    keep.append(inst)
                    continue
                # The block list is a global cross-engine order; only
                # PE-queue instructions are relevant between the halves.
                # PE EventSemaphores are safe to merge across: a weights
                # LOAD's only data hazard is the mk DMA, and both halves
                # come from the same single transfer the T0 load already
                # waits on.
                if (str(getattr(inst, "engine", None)) == "EngineType.PE"
                        and not isinstance(
                            inst,
                            (mybir.InstMatmult, mybir.InstEventSemaphore))):
                    pending = None
                keep.append(inst)
            blk.instructions[:] = keep


def _ldw_key(inst):
    ap = inst.ins[0]
    return repr(ap)


def _dedupe_ldweights(nc):
    """Drop an InstLdweights whose weights AP is identical to the
    immediately-preceding one (only InstMatmult in between): the a/b column
    halves of mm2 share one stationary operand, and a duplicate 256-col
    DoubleRow weight load would make the weight port the bottleneck. The
    dropped load's waits move to the surviving one (deduplicated)."""
    for fn in nc.m.functions:
        for blk in fn.blocks:
            keep = []
            last_ldw = None
            removed_any = False
            for inst in blk.instructions:
                if isinstance(inst, mybir.InstLdweights):
                    if (last_ldw is not None
                            and _ldw_key(inst) == _ldw_key(last_ldw[0])
                            and inst.perf_mode == last_ldw[0].perf_mode):
                        _merge_waits(last_ldw[0], inst)
                        removed_any = True
                        continue
                    last_ldw = (inst,)
                    keep.append(inst)
                    continue
                # Only PE-queue instructions can invalidate the loaded
                # weights; interleaved ACT/DVE/DMA entries in the global
                # block list don't. PE EventSemaphores are waits, safe to
                # dedupe across (the dropped load's waits move earlier).
                if (str(getattr(inst, "engine", None)) == "EngineType.PE"
                        and not isinstance(inst, mybir.InstEventSemaphore)
                        and not isinstance(inst, mybir.InstMatmult)):
                    last_ldw = None
                keep.append(inst)
            if removed_any:
                blk.instructions[:] = keep


def _strip_same_engine_waits(nc):
    """Drop redundant same-engine semaphore waits on ACT/PE compute
    instructions (each engine executes its queue in order, and TRN2 allows
    only one wait per instruction before EventSemaphore splitting)."""
    prefixes = {
        "EngineType.Activation": "Activation_",
        "EngineType.PE": "PE_",
    }
    kinds = (mybir.InstActivation, mybir.InstMatmult, mybir.InstLdweights)
    for fn in nc.m.functions:
        for blk in fn.blocks:
            for inst in blk.instructions:
                si = getattr(inst, "sync_info", None)
                if si is None or not si.on_wait or not isinstance(inst, kinds):
                    continue
                pref = prefixes.get(str(getattr(inst, "engine", None)))
                if pref is None:
                    continue
                kept = [w for w in si.on_wait
                        if not str(getattr(w, "ant_name", "")).startswith(pref)]
                if len(kept) != len(si.on_wait):
                    si.on_wait = kept


def _get_program():
    if "nc" not in _CACHE:
        _CACHE["nc"] = _build_program()
    return _CACHE["nc"]


def _make_in_maps(mk, mv, qk):
    f8 = ml_dtypes.float8_e4m3
    mkf = np.ascontiguousarray(mk.reshape(B, CK, N))
    mvf = np.ascontiguousarray(mv.reshape(B, CV, N))
    qkf = np.ascontiguousarray(qk.reshape(B, CK, P))
    in_maps = []
    for core in range(8):
        b, half = core // 2, core % 2
        n0, n1 = half * NHALF, (half + 1) * NHALF
        # mk blocks: [128, NB, 128]; partitions 0:64 = even tile's CK
        # channels, 64:128 = odd tile's (tile NT..pad = zeros).
        mk_c = np.zeros((CK, NTP * 128), dtype=f8)
        mk_c[:, :NHALF] = mkf[b, :, n0:n1].astype(f8)
        mk_pairs = mk_c.reshape(CK, NB, 2, 128)
        mk_t = np.zeros((128, NB, 128), dtype=f8)
        mk_t[:CK] = mk_pairs[:, :, 0, :]
        mk_t[CK:] = mk_pairs[:, :, 1, :]
        # qk duplicated into partitions 64:128 for the T8 row tile.
        qk_c = np.zeros((128, 2048), dtype=f8)
        qk_c[:CK, :P] = qkf[b].astype(f8)
        qk_c[CK:2 * CK, :P] = qk_c[:CK, :P]
        # mvT with the ones column at 512; zeros elsewhere (incl. pad rows
        # and pad tile NT..NTP so the DoubleRow partner contributes nothing)
        mvt = np.zeros((NTP * 128, MVW), dtype=f8)
        mvt[:NHALF, :CV] = mvf[b, :, n0:n1].T.astype(f8)
        mvt[:NHALF, CV] = 1.0
        mvt_c = np.ascontiguousarray(
            mvt.reshape(NTP, 128, MVW).transpose(1, 0, 2))
        in_maps.append({"mk": np.ascontiguousarray(mk_t),
                        "qk": np.ascontiguousarray(qk_c),
                        "mvT": mvt_c})
    return in_maps


def _run(mk, mv, qk, qv, trace=False, **spmd_kwargs):
    nc = _get_program()
    in_maps = _make_in_maps(mk, mv, qk)
    res = run_bass_kernel_spmd(nc, in_maps, list(range(8)), trace=trace,
                               **spmd_kwargs)
    out = np.empty((B, 2 * CV, P), dtype=np.float32)
    for b in range(B):
        m0 = res.results[2 * b]["memT"]
        m1 = res.results[2 * b + 1]["memT"]
        ms = m0 + m1
        out[b, :CV] = (ms[:, :CV] / ms[:, CV][:, None]).T
        out[b, CV:] = qv[b].reshape(CV, P)
    return out.reshape(B, 2 * CV, H, W), res


def kernel(mk, mv, qk, qv):
    out, _ = _run(np.asarray(mk), np.asarray(mv), np.asarray(qk),
                  np.asarray(qv))
    return out


# revision 7
# speedup vs baseline: 1.1271x; 1.1271x over previous
"""Trainium2 Bass kernel for nn_MemoryReader.

Reference computation (per batch b):
    mi = mk.reshape(CK, N);  qi = qk.reshape(CK, P) / sqrt(CK)
    S  = mi.T @ qi                      # [N, P] affinity logits
    A  = softmax(S, axis=0)             # over memory axis N
    mem = mv.reshape(CV, N) @ A         # [CV, P]
    out = concat([mem, qv], axis=channel)

Sharding: 8 cores = (4 batches) x (2 halves of the memory axis N).
Each core computes, for its (b, half):
    S      = mk.T @ qk                          # fp8 operands, f32 PSUM
    E      = exp(0.125*S - 2)                   # fp8e4; the -2 bias keeps
                                                # E<=54 (TRN fp8e4 max 240)
                                                # and cancels in the softmax
    memT   = E.T @ [mvT | 1]                    # [P, 513]: col 512 = sum(E)
The host combines: mem = (num_0 + num_1) / (den_0 + den_1), then concats
qv (pure passthrough). No on-device collectives needed.

Speed features (210.7us bf16 baseline -> 131.4us, all measured on HW):
  - mm1's contraction is only CK=64, padded to K=128 with zeros: full-row
    LDWEIGHTS go through the background weight buffer (K=64 loads
    serialize on the weight port). (PE 64x128 row tiling with concurrent
    T0/T8 pair matmuls was tried and measured: the pair matmuls DO run
    concurrently (2ns skew), but an LDWEIGHTS cannot background while
    TILED matmuls are in flight, so every pair pays foreground
    LDW + pipeline-drain latency: 303ns/pair vs 220ns for two plain
    full-array matmuls, plus tiled<->full transition stalls. Net +20us.)
  - mm2 (89% of FLOPs) runs in fp8e4 DoubleRow mode: each matmul
    contracts TWO 128-row n-tiles (lhsT/rhs get [K, 2, M] APs); measured
    ~110ns issue cadence per 256-col matmul with the DR LDWEIGHTS fully
    hidden by the background weight buffer.
  - The softmax denominator is a 513th "ones" column of mvT, accumulated
    by the same mm2 matmuls (split 256+257 to satisfy the one-PSUM-bank
    rule, sharing one LDWEIGHTS via a post-build dedupe pass) - no
    vector-engine accumulation at all.
  - exp() is fused 4 n-tiles per ONE ACT instruction (a 2-bank PSUM
    "s4" squad tile [128, 2, 2, 256]), amortizing the ~185ns ACT access
    latency: ACT busy 95.6us vs 106.8us with 2-tile fusion. e4 is laid
    out [128, 2(b), 2(i), 256] with n-tile q0+2i+b at slot (b, i), so a
    DoubleRow pair for mm2 is the slice e4[:, :, i, :] (Ko step 512B).
  - acc_a for the chunk's two p-slices shares ONE PSUM bank ([128, 2,
    256]); slice 1's first matmul uses start=False (slice 0's start
    already cleared the bank's has_written bits) - frees the bank needed
    by the s4 double buffer.
  - All inputs ship as fp8 in few, fat, consumption-ordered DMAs (2KB+
    per-partition rows; sub-2KB rows run ~5x slower).
  - A flat (chunk, quad) software pipeline issues unit u+1's mm1 before
    unit u's mm2, keeping the PE queue gapless.
"""

import numpy as np
import ml_dtypes

import concourse.tile as tile
from concourse import bacc, mybir
from concourse.bass_utils import run_bass_kernel_spmd

# Problem shape (hardcoded per contract)
B, CK, CV, T, H, W = 4, 64, 512, 8, 30, 54
N = T * H * W          # 12960 memory positions
P = H * W              # 1620 query positions
NHALF = N // 2         # 6480 per core
NT = (NHALF + 127) // 128   # 51 n-tiles (last has 80 rows)
NLAST = NHALF - (NT - 1) * 128  # 80
NTP = NT + 1           # pad to even tile count for DoubleRow pairing
MVW = 528              # mvT free width: 512 mv + 1 ones + 15 pad (16B align)
CVA = 513              # real mm2 output width (512 mv + 1 denominator)
ASPL = 256             # a-half columns (b-half = 257); each fits a PSUM bank
EXP_BIAS = -2.0        # exp(0.125*s - 2): range safety for fp8e4

# p-axis chunks of 256 (2 slices of 128 each; 84-wide remainder last — it
# has the smallest output-DMA tail. Running it first was tried and is 8%
# WORSE: its mm2 consumes one mvT tile per ~330ns, 2x what the DMA queues
# can sustain while the full mvT load is still in flight.)
PCH = [(0, 256), (256, 256), (512, 256), (768, 256), (1024, 256),
       (1280, 256), (1536, 84)]
QUADS = [(0, 4), (4, 4), (8, 4), (12, 4), (16, 4), (20, 4), (24, 4), (28, 4),
         (32, 4), (36, 4), (40, 4), (44, 4), (48, 3)]

DEDUPE_LDW = True      # share one LDW between mm2's a/b column halves
N_WARMUP = 40          # bridge PE preamble end (~7.1us) to data (~11us)

_CACHE = {}


def _build_program():
    f8 = mybir.dt.float8e4
    bf16 = mybir.dt.bfloat16
    f32 = mybir.dt.float32
    DR = mybir.MatmulPerfMode.DoubleRow
    nc = bacc.Bacc(None, target_bir_lowering=False, debug=False)

    # mk/qk zero-padded to K=128 on the host (see docstring).
    mk_d = nc.declare_dram_parameter("mk", [128, NT, 128], f8, isOutput=False)
    # padded to 2048 EXACTLY: the one qk transfer then has 2KB
    # per-partition rows. 1664 was tried and is 3us worse - sub-2KB rows
    # drop into the slow DMA class and delay every transfer behind them.
    qk_d = nc.declare_dram_parameter("qk", [128, 2048], f8, isOutput=False)
    mvt_d = nc.declare_dram_parameter("mvT", [128, NTP, MVW], f8, isOutput=False)
    mem_d = nc.declare_dram_parameter("memT", [P, CVA], f32, isOutput=True)

    with tile.TileContext(nc) as tc:
        with (
            tc.tile_pool(name="singles", bufs=1) as singles,
            tc.tile_pool(name="epool", bufs=3) as epool,
            tc.tile_pool(name="opool", bufs=4) as opool,
            tc.tile_pool(name="spsum", bufs=2, space="PSUM") as spsum,
            tc.tile_pool(name="apsum", bufs=2, space="PSUM") as apsum,
            tc.tile_pool(name="bpsum", bufs=2, space="PSUM") as bpsum,
        ):
            qk_sb = singles.tile([128, 2048], f8)
            mk_sb = singles.tile([128, NT, 128], f8)
            mvt_sb = singles.tile([128, NTP, MVW], f8)
            # DMA facts (measured): every transfer stripes over all 16
            # queues and completes in ISSUE ORDER at aggregate BW, but
            # sub-2KB per-partition rows run ~5x slower (per-row descriptor
            # overhead), and each trigger costs ~660ns on the Sync queue.
            # So: few, fat, consumption-ordered transfers.
            nc.sync.dma_start(out=qk_sb[:, :], in_=qk_d[:, :])
            mk_groups = [(0, 16), (16, 32), (32, NT)]
            mv_groups = [(g, min(g + 4, NTP)) for g in range(0, NTP, 4)]
            order = [("mk", 0), ("mv", 0), ("mv", 1), ("mv", 2), ("mk", 1),
                     ("mv", 3), ("mv", 4), ("mv", 5), ("mv", 6), ("mk", 2),
                     ("mv", 7), ("mv", 8), ("mv", 9), ("mv", 10), ("mv", 11),
                     ("mv", 12)]
            for kind, gi in order:
                if kind == "mk":
                    g0, g1 = mk_groups[gi]
                    nc.sync.dma_start(out=mk_sb[:, g0:g1, :],
                                      in_=mk_d[:, g0:g1, :])
                else:
                    g0, g1 = mv_groups[gi]
                    nc.sync.dma_start(out=mvt_sb[:, g0:g1, :],
                                      in_=mvt_d[:, g0:g1, :])

            # Warm-up: full-size matmuls on a memset tile, depending on no
            # DMA. They fill the initial PE idle gap AND release the HAM
            # clock throttle (~3.4us of sustained activity needed). Data
            # lands ~11us (DMA ramp starts ~8.7us); cold MMs are ~107ns.
            warmw = singles.tile([128, 128], bf16, name="warmw")
            nc.vector.memset(warmw, 1.0)
            bias_sb = singles.tile([128, 1], f32, name="bias")
            nc.vector.memset(bias_sb, EXP_BIAS)
            warm = spsum.tile([128, 2, 2, ASPL], f32, tag="s4", name="warm")
            for _ in range(N_WARMUP):
                nc.tensor.matmul(warm[:, 0, 0, 0:128], lhsT=warmw, rhs=warmw,
                                 start=True, stop=True)

            # Flat software pipeline over (chunk, quad) units: issue unit
            # u+1's mm1 before unit u's mm2 so the PE queue never stalls on
            # the ACT->mm2 dependency.
            units = [(ci, qi) for ci in range(len(PCH)) for qi in range(len(QUADS))]
            squads = {}
            e4s = {}
            accs = {}

            def issue_mm1(u):
                ci, qi = units[u]
                ps, w = PCH[ci]
                q0, qn = QUADS[qi]
                # s4 squad tile [128, b, i, 256]: n-tile q0+2i+b goes to
                # slot (b, i) — bank b holds pair i's two tiles split so
                # that a DoubleRow partner pair is s4[:, :, i, :].
                s4 = spsum.tile([128, 2, 2, ASPL], f32, tag="s4", name="s4")
                for j in range(qn):
                    nt = q0 + j
                    nsz = 128 if nt < NT - 1 else NLAST
                    nc.tensor.matmul(
                        s4[:nsz, j % 2, j // 2, :w],
                        lhsT=mk_sb[:, nt, :nsz],
                        rhs=qk_sb[:, ps:ps + w],
                        start=True,
                        stop=True,
                    )
                squads[u] = s4

            def issue_act(u):
                ci, qi = units[u]
                ps, w = PCH[ci]
                q0, qn = QUADS[qi]
                # One fused ACT per quad (4 n-tiles, 2 PSUM banks). For the
                # 3-tile last quad, slot (1,1) is exp of stale PSUM — mm2
                # never reads it (same trick as rows 80:128 of tile 50).
                e4 = epool.tile([128, 2, 2, ASPL], f8, tag="e", name="e")
                s4 = squads.pop(u)
                nc.scalar.activation(
                    out=e4[:, :, :, :w],
                    in_=s4[:, :, :, :w],
                    func=mybir.ActivationFunctionType.Exp,
                    scale=0.125,  # 1/sqrt(CK)
                    bias=bias_sb[:, :],
                )
                e4s[u] = e4

            def issue_mm2(u):
                ci, qi = units[u]
                ps, w = PCH[ci]
                q0, qn = QUADS[qi]
                e4 = e4s.pop(u)
                nslices = (w + 127) // 128
                if qi == 0:
                    # acc_a: ONE bank for both p-slices ([128, 2, 256]).
                    # acc_b: 257 cols per slice won't pair in a bank.
                    accs[ci] = (
                        apsum.tile([128, 2, ASPL], f32, tag="acc_a",
                                   name="acc_a"),
                        [bpsum.tile([128, 512], f32, tag="acc_b", name="acc_b")
                         for _ in range(nslices)],
                    )
                first = qi == 0
                last = qi == len(QUADS) - 1

                def mm2_step(i, dr, sl, st, sp):
                    nt = q0 + 2 * i
                    nsz = 128 if dr else NLAST
                    pw = min(128, w - 128 * sl)
                    acc_a, acc_bs = accs[ci]
                    if dr:
                        el = e4[:nsz, :, i, sl * 128:sl * 128 + pw]
                        nc.tensor.matmul(
                            acc_a[:pw, sl, 0:ASPL],
                            lhsT=el,
                            rhs=mvt_sb[:nsz, nt:nt + 2, 0:ASPL],
                            # slice 1 shares slice 0's bank: slice 0's
                            # start already cleared the bank's has_written
                            # bits, so slice 1 must NOT clear them again.
                            start=st and sl == 0, stop=sp,
                            perf_mode=DR,
                        )
                        nc.tensor.matmul(
                            acc_bs[sl][:pw, 0:CVA - ASPL],
                            lhsT=el,
                            rhs=mvt_sb[:nsz, nt:nt + 2, ASPL:CVA],
                            start=st, stop=sp,
                            perf_mode=DR,
                        )
                    else:
                        el = e4[:nsz, 0, i, sl * 128:sl * 128 + pw]
                        nc.tensor.matmul(
                            acc_a[:pw, sl, 0:ASPL],
                            lhsT=el,
                            rhs=mvt_sb[:nsz, nt, 0:ASPL],
                            start=st and sl == 0, stop=sp,
                        )
                        nc.tensor.matmul(
                            acc_bs[sl][:pw, 0:CVA - ASPL],
                            lhsT=el,
                            rhs=mvt_sb[:nsz, nt, ASPL:CVA],
                            start=st, stop=sp,
                        )

                def copy_out(sl, on_act):
                    pw = min(128, w - 128 * sl)
                    acc_a, acc_bs = accs[ci]
                    o_sb = opool.tile([128, CVA], f32, tag="o", name="o")
                    nc.vector.tensor_copy(out=o_sb[:pw, 0:ASPL],
                                          in_=acc_a[:pw, sl, 0:ASPL])
                    if on_act:
                        # ACT is idle at the kernel tail; mid-stream ACT
                        # copies delay the next chunk's exp (~400ns/bound).
                        # (gpsimd PSUM reads crash at runtime.)
                        nc.scalar.activation(
                            out=o_sb[:pw, ASPL:CVA],
                            in_=acc_bs[sl][:pw, 0:CVA - ASPL],
                            func=mybir.ActivationFunctionType.Copy,
                        )
                    else:
                        nc.vector.tensor_copy(out=o_sb[:pw, ASPL:CVA],
                                              in_=acc_bs[sl][:pw, 0:CVA - ASPL])
                    p0 = ps + sl * 128
                    nc.sync.dma_start(out=mem_d[p0:p0 + pw, :],
                                      in_=o_sb[:pw, :])

                # DR pairs over (i=0: tiles q0,q0+1) and (i=1: q0+2,q0+3);
                # the 3-tile last quad does DR(48,49) then single(50).
                if qn == 4:
                    steps = [(0, True), (1, True)]
                else:
                    steps = [(0, True), (1, False)]
                for si, (i, dr) in enumerate(steps):
                    st = first and si == 0
                    sp = last and si == len(steps) - 1
                    for sl in range(nslices):
                        mm2_step(i, dr, sl, st, sp)
                if last:
                    final = ci == len(PCH) - 1
                    for sl in range(nslices):
                        copy_out(sl, on_act=final)
                    del accs[ci]

            issue_mm1(0)
            for u in range(len(units)):
                if u + 1 < len(units):
                    issue_mm1(u + 1)
                issue_act(u)
                issue_mm2(u)

    _strip_same_engine_waits(nc)
    if DEDUPE_LDW:
        _dedupe_ldweights(nc)
    nc.compile()
    return nc


def _merge_waits(keep, dropped):
    si = getattr(dropped, "sync_info", None)
    if si is not None and si.on_wait:
        ksi = keep.sync_info
        if ksi is None:
            keep.sync_info = si
        else:
            have = {repr(w) for w in ksi.on_wait}
            for w_ in si.on_wait:
                if repr(w_) not in have:
                    ksi.on_wait.append(w_)
        assert not (si.on_update or []), "dropped LDW had sem updates"


def _ldw_key(inst):
    ap = inst.ins[0]
    return repr(ap)


def _dedupe_ldweights(nc):
    """Drop an InstLdweights whose weights AP is identical to the
    immediately-preceding one (only PE matmuls / event semaphores in
    between): the a/b column halves of mm2 share one stationary operand,
    and a duplicate 256-col DoubleRow weight load would make the weight
    port the bottleneck. The dropped load's waits move to the surviving
    one (deduplicated)."""
    for fn in nc.m.functions:
        for blk in fn.blocks:
            keep = []
            last_ldw = None
            removed_any = False
            for inst in blk.instructions:
                if isinstance(inst, mybir.InstLdweights):
                    if (last_ldw is not None
                            and _ldw_key(inst) == _ldw_key(last_ldw[0])
                            and inst.perf_mode == last_ldw[0].perf_mode):
                        _merge_waits(last_ldw[0], inst)
                        removed_any = True
                        continue
                    last_ldw = (inst,)
                    keep.append(inst)
                    continue
                # Only PE-queue instructions can invalidate the loaded
                # weights; interleaved ACT/DVE/DMA entries in the global
                # block list don't. PE EventSemaphores are waits, safe to
                # dedupe across (the dropped load's waits move earlier).
                if (str(getattr(inst, "engine", None)) == "EngineType.PE"
                        and not isinstance(inst, mybir.InstEventSemaphore)
                        and not isinstance(inst, mybir.InstMatmult)):
                    last_ldw = None
                keep.append(inst)
            if removed_any:
                blk.instructions[:] = keep


def _strip_same_engine_waits(nc):
    """Drop redundant same-engine semaphore waits on ACT/PE compute
    instructions (each engine executes its queue in order, and TRN2 allows
    only one wait per instruction before EventSemaphore splitting)."""
    prefixes = {
        "EngineType.Activation": "Activation_",
        "EngineType.PE": "PE_",
    }
    kinds = (mybir.InstActivation, mybir.InstMatmult, mybir.InstLdweights)
    for fn in nc.m.functions:
        for blk in fn.blocks:
            for inst in blk.instructions:
                si = getattr(inst, "sync_info", None)
                if si is None or not si.on_wait or not isinstance(inst, kinds):
                    continue
                pref = prefixes.get(str(getattr(inst, "engine", None)))
                if pref is None:
                    continue
                kept = [w for w in si.on_wait
                        if not str(getattr(w, "ant_name", "")).startswith(pref)]
                if len(kept) != len(si.on_wait):
                    si.on_wait = kept


def _get_program():
    if "nc" not in _CACHE:
        _CACHE["nc"] = _build_program()
    return _CACHE["nc"]


def _make_in_maps(mk, mv, qk):
    f8 = ml_dtypes.float8_e4m3
    mkf = np.ascontiguousarray(mk.reshape(B, CK, N))
    mvf = np.ascontiguousarray(mv.reshape(B, CV, N))
    qkf = np.ascontiguousarray(qk.reshape(B, CK, P))
    in_maps = []
    for core in range(8):
        b, half = core // 2, core % 2
        n0, n1 = half * NHALF, (half + 1) * NHALF
        mk_c = mkf[b, :, n0:n1].astype(f8)             # [64, 6480]
        mk_t = np.zeros((128, NT, 128), dtype=f8)
        mk_t[:CK].reshape(CK, NT * 128)[:, :NHALF] = mk_c
        qk_c = np.zeros((128, 2048), dtype=f8)
        qk_c[:CK, :P] = qkf[b].astype(f8)
        # mvT with the ones column at 512; zeros elsewhere (incl. pad rows
        # and pad tile NT..NTP so the DoubleRow partner contributes nothing)
        mvt = np.zeros((NTP * 128, MVW), dtype=f8)
        mvt[:NHALF, :CV] = mvf[b, :, n0:n1].T.astype(f8)
        mvt[:NHALF, CV] = 1.0
        mvt_c = np.ascontiguousarray(
            mvt.reshape(NTP, 128, MVW).transpose(1, 0, 2))
        in_maps.append({"mk": np.ascontiguousarray(mk_t),
                        "qk": np.ascontiguousarray(qk_c),
                        "mvT": mvt_c})
    return in_maps


def _run(mk, mv, qk, qv, trace=False, **spmd_kwargs):
    nc = _get_program()
    in_maps = _make_in_maps(mk, mv, qk)
    res = run_bass_kernel_spmd(nc, in_maps, list(range(8)), trace=trace,
                               **spmd_kwargs)
    out = np.empty((B, 2 * CV, P), dtype=np.float32)
    for b in range(B):
        m0 = res.results[2 * b]["memT"]
        m1 = res.results[2 * b + 1]["memT"]
        ms = m0 + m1
        out[b, :CV] = (ms[:, :CV] / ms[:, CV][:, None]).T
        out[b, CV:] = qv[b].reshape(CV, P)
    return out.reshape(B, 2 * CV, H, W), res


def kernel(mk, mv, qk, qv):
    out, _ = _run(np.asarray(mk), np.asarray(mv), np.asarray(qk),
                  np.asarray(qv))
    return out
